# revision 44
# baseline (speedup 1.0000x reference)
"""Trainium2 Bass kernel for the GCN autoencoder problem.

kernel(**inputs) takes the FULL unsharded inputs (x, edge_index, W, b, gamma,
beta), distributes across 8 NeuronCores internally, and returns the full
[12000, 12000] float32 output of:
  GCNConv (self-loops, symmetric norm) -> BatchNorm1d -> ReLU -> z @ z.T

Strategy (v4, dense-adjacency): the scatter-add is reformulated as a dense
matmul hT = xw'.T @ M where M[s, d] is the edge-multiplicity matrix shipped
as fp8_e4m3 (small integers -> exact).  The symmetric normalization
dinv[s]*dinv[d] is folded into xw' (src side, per-partition scale) and a
post-matmul column scale (dst side).  Each core owns 1536 dst nodes.  BN
batch statistics are computed per-core during conv and shipped as 2 extra
fp16 AllGather payload columns.  The AllGather is split: chunks 0-1 are
gathered while conv chunk 2 still runs (hiding the collective launch), then
a small chunk-2+stats gather follows; ReLU is applied in place per gathered
slice (ACT + DVE), pipelined behind the readback DMAs.  The z @ z.T decode
runs a block-tournament so only ~54% of the symmetric output is computed and
written (fp16, paired 2-bank PSUM copies, output DMAs alternating across
both HWDGE rings); the host mirrors the rest.

Self-contained: only needs numpy + ml_dtypes + the concourse (Bass) runtime.
"""

import numpy as np
import ml_dtypes

import concourse.bass as bass
import concourse.bacc as bacc
import concourse.mybir as mybir
import concourse.tile as tile

N = 12000
C_IN = 256
F = 128
P = 128
NCORES = 8
NW = 12                      # 128-blocks per core (rows)
NLOC = NW * P                # 1536 nodes per shard (padded)
NPAD = NCORES * NLOC         # 12288
NBLK = NPAD // P             # 96 blocks of 128
SBLK = 94                    # non-pad src blocks (94,95 are all-zero pad)
TPC = 8                      # blocks per residue class (96/12)
NQ = 3                       # dst chunks per core
DQ = 512
SBATCH = 12                  # src-blocks per A DMA batch
BN_EPS = 1e-5
AT_COLS = NQ * NBLK * DQ     # 147456
WAG = NLOC + 2               # AllGather payload width (h chunk + 2 stat cols)
BN_SCALE = 16.0              # stats carried as sum/16 to stay in fp16 range

# residue-class tournament: C_k = classes whose columns block-row-slot k computes
CLS = []
for k in range(NW):
    cs = [k] + [(k + d) % NW for d in range(1, 6)]
    if k < 6:
        cs.append((k + 6) % NW)
    CLS.append(sorted(cs))
SLOT_W = [len(c) * TPC * P for c in CLS]          # output cols per slot
SLOT_OFF = np.concatenate([[0], np.cumsum(SLOT_W)]).astype(int)
OUT_W = int(SLOT_OFF[-1])                          # 79872

AF = mybir.ActivationFunctionType
ALU = mybir.AluOpType


# --------------------------------------------------------------------------
# Host-side preprocessing: indices -> dense fp8 multiplicity matrix + scales.
# --------------------------------------------------------------------------

def preprocess(x, edge_index, W, gamma, beta):
    src = np.asarray(edge_index[0]).astype(np.int64)
    dst = np.asarray(edge_index[1]).astype(np.int64)
    src_all = np.concatenate([src, np.arange(N, dtype=np.int64)])
    dst_all = np.concatenate([dst, np.arange(N, dtype=np.int64)])
    deg = np.bincount(dst_all, minlength=N).astype(np.float32)
    dinv_pad = np.ones(NPAD, dtype=np.float32)
    dinv_pad[:N] = 1.0 / np.sqrt(deg)

    counts = np.zeros((NPAD, NPAD), dtype=np.uint8)
    np.add.at(counts, (src_all, dst_all), 1)
    M8 = counts.astype(ml_dtypes.float8_e4m3)     # exact small ints

    # fold the src-side dinv into x rows (commutes with @W): xw' = (dinv*x)@W
    xpad = np.zeros((NPAD, C_IN), dtype=np.float16)
    xpad[:N] = (np.asarray(x, dtype=np.float32)
                * dinv_pad[:N, None]).astype(np.float16)
    # pack x^T so each 12-block group is one contiguous per-partition DMA:
    # xT[p, (g*2+half)*1536 + col] = x[g*1536+col, half*128+p]
    xT = np.ascontiguousarray(
        xpad.T.reshape(2, P, 8, NLOC).transpose(1, 2, 0, 3)
        .reshape(P, 2 * NPAD))
    W16 = np.asarray(W, dtype=np.float32).astype(np.float16)  # [256, 128]
    gamma2 = np.asarray(gamma, dtype=np.float32).reshape(F, 1)
    beta2 = np.asarray(beta, dtype=np.float32).reshape(F, 1)

    in_maps = []
    for c in range(NCORES):
        Ml = M8[:, c * NLOC:(c + 1) * NLOC]
        A_packed = np.ascontiguousarray(
            Ml.reshape(NBLK, P, NQ, DQ).transpose(1, 2, 0, 3)
              .reshape(P, AT_COLS))
        dinvd = np.ascontiguousarray(np.broadcast_to(
            dinv_pad[c * NLOC:(c + 1) * NLOC], (P, NLOC)).astype(np.float16))
        in_maps.append({
            "xT": xT,
            "Wt": W16,
            "gamma": gamma2,
            "beta": beta2,
            "A_packed": A_packed,
            "dinvd": dinvd,
        })
    return in_maps, None


# --------------------------------------------------------------------------
# Device program (one SPMD program for all 8 cores).
# --------------------------------------------------------------------------

def build(meta=None, bench_phase=None, bench_r=8, repeat=1, upto=4,
          n_pool_relu=2, dec_half_outer=True, split_ag=True,
          out_ring_split=True):
    nc = bacc.Bacc("TRN2", target_bir_lowering=False, debug=False,
                   num_devices=NCORES)
    f32 = mybir.dt.float32
    fp16 = mybir.dt.float16
    fp8 = mybir.dt.float8e4

    xT_d = nc.dram_tensor("xT", [P, 2 * NPAD], fp16, kind="ExternalInput")
    Wt_d = nc.dram_tensor("Wt", [C_IN, F], fp16, kind="ExternalInput")
    gamma_d = nc.dram_tensor("gamma", [F, 1], f32, kind="ExternalInput")
    beta_d = nc.dram_tensor("beta", [F, 1], f32, kind="ExternalInput")
    A_d = nc.dram_tensor("A_packed", [P, AT_COLS], fp8, kind="ExternalInput")
    dinvd_d = nc.dram_tensor("dinvd", [P, NLOC], fp16, kind="ExternalInput")
    out_d = nc.dram_tensor("out", [P, OUT_W], fp16, kind="ExternalOutput")

    rg = [list(range(NCORES))]

    with tile.TileContext(nc) as tc:
      for rep in range(repeat):
        with tc.tile_pool(name="const", bufs=1) as const, \
             tc.tile_pool(name="big", bufs=1) as big, \
             tc.tile_pool(name="px", bufs=2) as px, \
             tc.tile_pool(name="pa", bufs=6) as pa, \
             tc.tile_pool(name="pb", bufs=1) as pb, \
             tc.tile_pool(name="pd", bufs=3) as pd, \
             tc.tile_pool(name="dram", bufs=1, space="DRAM") as dram:
            # conv PSUM pools (pxps/pcps) and the decode PSUM pool (pdps)
            # are opened in disjoint scopes so decode gets 8 banks
            pools = {}
            # ---------------- constants ----------------
            W_sb = const.tile([P, 2 * F], fp16)
            nc.sync.dma_start(W_sb[:, 0:F], Wt_d[0:P, :])
            nc.sync.dma_start(W_sb[:, F:2 * F], Wt_d[P:2 * P, :])
            gamma_sb = const.tile([P, 1], f32)
            nc.sync.dma_start(gamma_sb[:], gamma_d[:, :])
            beta_sb = const.tile([P, 1], f32)
            nc.sync.dma_start(beta_sb[:], beta_d[:, :])
            dinvd_sb = const.tile([P, NLOC], fp16)
            nc.sync.dma_start(dinvd_sb[:], dinvd_d[:, :])
            eps_sb = const.tile([P, 1], f32)
            nc.gpsimd.memset(eps_sb[:], BN_EPS)

            # persistent tiles
            xw_all = big.tile([P, NPAD], fp16)      # xw' blocks, [s-part, f]
            hT_loc = big.tile([P, NLOC], fp16)      # local h, feature-major
            hzTs = big.tile([P, NCORES * WAG], fp16)  # gathered h (+stats)
            zT_loc = big.tile([P, NLOC], fp16)
            sq_scr = big.tile([P, NLOC], fp16)      # Square / affine scratch
            ssum_p = big.tile([P, NQ], f32)
            ssq_p = big.tile([P, NQ], f32)
            spack = big.tile([P, 2], fp16)
            stat_rb = big.tile([P, 2 * NCORES], fp16)

            agi1 = dram.tile([P, WAG], fp16, name=f"agione_{rep}")
            ago1 = dram.tile([NCORES * P, WAG], fp16, addr_space="Shared",
                             name=f"agoone_{rep}")
            if split_ag:
                agiA = dram.tile([P, 2 * DQ], fp16, name=f"agia_{rep}")
                agoA = dram.tile([NCORES * P, 2 * DQ], fp16,
                                 addr_space="Shared", name=f"agoa_{rep}")
                agiB = dram.tile([P, DQ + 2], fp16, name=f"agib_{rep}")
                agoB = dram.tile([NCORES * P, DQ + 2], fp16,
                                 addr_space="Shared", name=f"agob_{rep}")

            def ag_a():
                # gather chunks 0,1 while conv chunk 2 still runs
                nc.gpsimd.collective_compute(
                    "AllGather", ALU.bypass, replica_groups=rg,
                    ins=[agiA.opt()], outs=[agoA.opt()])
                for r in range(NCORES):
                    nc.sync.dma_start(hzTs[:, r * WAG:r * WAG + 2 * DQ],
                                      agoA[r * P:(r + 1) * P, :])

            hz_v = hzTs[:].rearrange("p (r w) -> p r w", r=NCORES)
            noag = bench_phase in ("noag", "fullnoag")

            # ------- phase X: xw' = (dinv[s]*x) @ W, all 96 blocks ----------
            def xw_group(g):
                xs = px.tile([P, 2 * NLOC], fp16, tag="xs")
                # ACT ring: keeps the sync ring free for the A-stream
                nc.scalar.dma_start(
                    xs[:], xT_d[:, 2 * g * NLOC:2 * (g + 1) * NLOC])
                for bq in range(NW // 4):       # 4 blocks per PSUM tile
                    xwps = pools['pxps'].tile([P, 4 * F], f32, tag="xwps")
                    for j in range(4):
                        b = bq * 4 + j
                        nc.tensor.matmul(xwps[:, j * F:(j + 1) * F],
                                         lhsT=xs[:, b * P:(b + 1) * P],
                                         rhs=W_sb[:, 0:F],
                                         start=True, stop=False)
                        nc.tensor.matmul(xwps[:, j * F:(j + 1) * F],
                                         lhsT=xs[:, NLOC + b * P:
                                                 NLOC + (b + 1) * P],
                                         rhs=W_sb[:, F:2 * F],
                                         start=False, stop=True)
                    s0 = (g * NW + bq * 4) * P
                    if bq % 2 == 0:
                        nc.scalar.copy(xw_all[:, s0:s0 + 4 * P], xwps[:])
                    else:
                        nc.vector.tensor_copy(xw_all[:, s0:s0 + 4 * P],
                                              xwps[:])

            def phasex():
                for g in range(8):
                    xw_group(g)

            # per-chunk epilogue: scale by dinv[dst], local BN stats, AG stage
            def chunk_epilogue(q, hps):
                nc.vector.tensor_tensor(
                    out=hT_loc[:, q * DQ:(q + 1) * DQ], in0=hps[:],
                    in1=dinvd_sb[:, q * DQ:(q + 1) * DQ], op=ALU.mult)
                nc.vector.reduce_sum(out=ssum_p[:, q:q + 1],
                                     in_=hT_loc[:, q * DQ:(q + 1) * DQ],
                                     axis=mybir.AxisListType.X)
                nc.scalar.activation(sq_scr[:, 0:DQ],
                                     hT_loc[:, q * DQ:(q + 1) * DQ],
                                     AF.Square,
                                     accum_out=ssq_p[:, q:q + 1])
                # stage this chunk of the AG input while conv continues
                if split_ag:
                    dst = (agiA[:, q * DQ:(q + 1) * DQ] if q < 2
                           else agiB[:, 0:DQ])
                else:
                    dst = agi1[:, q * DQ:(q + 1) * DQ]
                nc.scalar.dma_start(dst, hT_loc[:, q * DQ:(q + 1) * DQ])

            # ------- phase C: hT = xw'.T @ M, dst-chunked ------------------
            def conv_chunk(q):
                hps = pools['pcps'].tile([P, DQ], f32, tag="hps")
                for batch in range(NBLK // SBATCH):
                    nsb = min(SBATCH, SBLK - batch * SBATCH)
                    asb = pa.tile([P, SBATCH * DQ], fp8, tag="asb")
                    off = (q * NBLK + batch * SBATCH) * DQ
                    nc.sync.dma_start(asb[:, 0:nsb * DQ],
                                      A_d[:, off:off + nsb * DQ])
                    for j in range(nsb):
                        s = batch * SBATCH + j
                        nc.tensor.matmul(hps[:],
                                         lhsT=xw_all[:, s * P:(s + 1) * P],
                                         rhs=asb[:, j * DQ:(j + 1) * DQ],
                                         start=(s == 0), stop=(s == SBLK - 1))
                chunk_epilogue(q, hps)

            def merged_x_conv0():
                # interleave xw production (per 12-block group) with conv q=0
                # matmuls on the just-produced blocks; A q0 batch g aligns.
                hps = pools['pcps'].tile([P, DQ], f32, tag="hps")
                for g in range(8):
                    xw_group(g)
                    nsb = min(SBATCH, SBLK - g * SBATCH)
                    asb = pa.tile([P, SBATCH * DQ], fp8, tag="asb")
                    off = g * SBATCH * DQ
                    nc.sync.dma_start(asb[:, 0:nsb * DQ],
                                      A_d[:, off:off + nsb * DQ])
                    for j in range(nsb):
                        s = g * SBATCH + j
                        nc.tensor.matmul(hps[:],
                                         lhsT=xw_all[:, s * P:(s + 1) * P],
                                         rhs=asb[:, j * DQ:(j + 1) * DQ],
                                         start=(s == 0), stop=(s == SBLK - 1))
                chunk_epilogue(0, hps)

            # ------- AG + BN: gather h (+stats), relu per slice ------------
            def ag_bn():
                # fold 3-chunk partial stats, append to the AG payload
                # (sums of ~1536 fp16 values; 5e-4 fp16 relative error is
                # well inside the BN-stat tolerance)
                with nc.allow_low_precision(reason="fp16 AG stat payload"):
                    nc.vector.reduce_sum(out=spack[:, 0:1], in_=ssum_p[:],
                                         axis=mybir.AxisListType.X)
                    nc.vector.reduce_sum(out=spack[:, 1:2], in_=ssq_p[:],
                                         axis=mybir.AxisListType.X)
                    nc.vector.tensor_scalar_mul(spack[:], spack[:],
                                                1.0 / BN_SCALE)
                if split_ag:
                    nc.scalar.dma_start(agiB[:, DQ:DQ + 2], spack[:])
                    nc.gpsimd.collective_compute(
                        "AllGather", ALU.bypass, replica_groups=rg,
                        ins=[agiB.opt()], outs=[agoB.opt()])
                    for r in range(NCORES):
                        nc.sync.dma_start(
                            hzTs[:, r * WAG + 2 * DQ:(r + 1) * WAG],
                            agoB[r * P:(r + 1) * P, :])
                elif noag:
                    nc.scalar.dma_start(agi1[:, NLOC:WAG], spack[:])
                    nc.gpsimd.memset(stat_rb[:], 0.0)
                    for r in range(NCORES):
                        nc.sync.dma_start(hzTs[:, r * WAG:(r + 1) * WAG],
                                          agi1[:, :])
                else:
                    nc.scalar.dma_start(agi1[:, NLOC:WAG], spack[:])
                    nc.gpsimd.collective_compute(
                        "AllGather", ALU.bypass, replica_groups=rg,
                        ins=[agi1.opt()], outs=[ago1.opt()])
                    for r in range(NCORES):
                        nc.sync.dma_start(hzTs[:, r * WAG:(r + 1) * WAG],
                                          ago1[r * P:(r + 1) * P, :])
                # global stats: sum the 8 gathered (ssum, ssq) pairs
                ssum = pb.tile([P, 1], f32, tag="ssum")
                nc.vector.reduce_sum(out=ssum[:],
                                     in_=hz_v[:, :, NLOC:NLOC + 1],
                                     axis=mybir.AxisListType.XY)
                ssq = pb.tile([P, 1], f32, tag="ssq")
                nc.vector.reduce_sum(out=ssq[:],
                                     in_=hz_v[:, :, NLOC + 1:NLOC + 2],
                                     axis=mybir.AxisListType.XY)
                mean = pb.tile([P, 1], f32, tag="mean")
                nc.vector.tensor_scalar_mul(mean[:], ssum[:], BN_SCALE / N)
                ex2 = pb.tile([P, 1], f32, tag="ex2")
                nc.vector.tensor_scalar_mul(ex2[:], ssq[:], BN_SCALE / N)
                m2 = pb.tile([P, 1], f32, tag="m2")
                nc.vector.tensor_mul(m2[:], mean[:], mean[:])
                var = pb.tile([P, 1], f32, tag="var")
                nc.vector.tensor_tensor(out=var[:], in0=ex2[:], in1=m2[:],
                                        op=ALU.subtract)
                sd = pb.tile([P, 1], f32, tag="sd")
                nc.scalar.activation(sd[:], var[:], AF.Sqrt,
                                     bias=eps_sb[:, :1])
                rstd = pb.tile([P, 1], f32, tag="rstd")
                nc.vector.reciprocal(rstd[:], sd[:])
                scale_f = pb.tile([P, 1], f32, tag="scalef")
                nc.vector.tensor_mul(scale_f[:], rstd[:], gamma_sb[:])
                msc = pb.tile([P, 1], f32, tag="msc")
                nc.vector.tensor_mul(msc[:], mean[:], scale_f[:])
                shift_f = pb.tile([P, 1], f32, tag="shiftf")
                nc.vector.tensor_tensor(out=shift_f[:], in0=beta_sb[:],
                                        in1=msc[:], op=ALU.subtract)
                # z_loc = relu(scale*h_loc + shift) on DVE (2 passes) so the
                # ACT queue stays free for the per-slice relu pipeline
                nc.vector.tensor_scalar(out=sq_scr[:], in0=hT_loc[:],
                                        scalar1=scale_f[:, 0:1],
                                        scalar2=shift_f[:, 0:1],
                                        op0=ALU.mult, op1=ALU.add)
                nc.vector.tensor_scalar_max(zT_loc[:], sq_scr[:], 0.0)
                # relu each gathered slice in place, pipelined behind readback
                # (ACT does the first 6; Pool the last 2 so ACT frees up for
                # the decode copies sooner)
                for r in range(NCORES):
                    sl = hzTs[:, r * WAG:r * WAG + NLOC]
                    if r < NCORES - n_pool_relu:
                        nc.scalar.activation(sl, sl, AF.Relu,
                                             bias=shift_f[:, :1],
                                             scale=scale_f[:, :1])
                    else:
                        # DVE 2-pass affine+relu for the tail slices
                        nc.vector.tensor_scalar(out=sl, in0=sl,
                                                scalar1=scale_f[:, 0:1],
                                                scalar2=shift_f[:, 0:1],
                                                op0=ALU.mult, op1=ALU.add)
                        nc.vector.tensor_scalar_max(sl, sl, 0.0)

            # ---------------- phase D: decode z @ z.T (upper classes) --------
            # ob column layout: for half: for ci: one 512-col block at
            # half*(len(CLS[k])*512) + ci*512 (t-minor within each block).
            # Pairs of consecutive matmuls share one 2-bank PSUM tile and one
            # [P, 1024] copy to halve per-copy fixed overhead.
            def phased():
                cnt = 0
                for k in range(NW):
                    ncls = len(CLS[k])
                    w2 = ncls * 512
                    ob = pd.tile([P, max(SLOT_W)], fp16, tag="ob")
                    pairs = [(half, ci, m) for half in range(2)
                             for ci, m in enumerate(CLS[k])]
                    i = 0
                    while i < len(pairs):
                        group = pairs[i:i + 2]
                        ops = pools['pdps'].tile([P, 1024], f32, tag="ops")
                        for gj, (half, ci, m) in enumerate(group):
                            rhs = hz_v[:, half * 4:(half + 1) * 4,
                                       m * P:(m + 1) * P]
                            nc.tensor.matmul(
                                ops[:, gj * 512:(gj + 1) * 512],
                                lhsT=zT_loc[:, k * P:(k + 1) * P],
                                rhs=rhs, start=True, stop=True)
                        half, ci, m = group[0]
                        o0 = half * w2 + ci * 512
                        ow = 512 * len(group)
                        # the pair may wrap from half 0 into half 1 (odd ncls)
                        contig = (len(group) == 1 or
                                  (group[1][0] == half and group[1][1] == ci + 1)
                                  or (group[1][0] == half + 1 and
                                      group[1][1] == 0 and ci == ncls - 1))
                        assert contig
                        if cnt % 2 == 0:
                            nc.vector.tensor_copy(ob[:, o0:o0 + ow],
                                                  ops[:, 0:ow])
                        else:
                            nc.scalar.copy(ob[:, o0:o0 + ow], ops[:, 0:ow])
                        cnt += 1
                        i += len(group)
                    oeng = nc.scalar if (out_ring_split and k % 2) else nc.sync
                    oeng.dma_start(
                        out_d[:, int(SLOT_OFF[k]):int(SLOT_OFF[k]) + SLOT_W[k]],
                        ob[:, :SLOT_W[k]])

            def conv_pools():
                return tc.tile_pool(name="pxps", bufs=2, space="PSUM")

            def conv_pools2():
                return tc.tile_pool(name="pcps", bufs=2, space="PSUM")

            def dec_pool():
                return tc.tile_pool(name="pdps", bufs=4, space="PSUM")

            def whole():
                with conv_pools() as a, conv_pools2() as b:
                    pools["pxps"], pools["pcps"] = a, b
                    merged_x_conv0()
                    conv_chunk(1)
                    if split_ag:
                        ag_a()      # launch hides under conv chunk 2
                    conv_chunk(2)
                    ag_bn()
                with dec_pool() as c:
                    pools["pdps"] = c
                    phased()

            if bench_phase == "xw":
                with conv_pools() as a:
                    pools["pxps"] = a
                    with tc.For_i(0, bench_r, 1):
                        phasex()
            elif bench_phase == "conv":
                with conv_pools() as a, conv_pools2() as b:
                    pools["pxps"], pools["pcps"] = a, b
                    phasex()
                    with tc.For_i(0, bench_r, 1):
                        for q in range(NQ):
                            conv_chunk(q)
            elif bench_phase == "dec":
                with conv_pools() as a, conv_pools2() as b:
                    pools["pxps"], pools["pcps"] = a, b
                    phasex()
                    for q in range(NQ):
                        conv_chunk(q)
                    ag_bn()
                with dec_pool() as c:
                    pools["pdps"] = c
                    with tc.For_i(0, bench_r, 1):
                        phased()
            elif bench_phase == "fullnoag":
                with tc.For_i(0, bench_r, 1):
                    whole()
            else:
                if upto == 1:
                    with conv_pools() as a:
                        pools["pxps"] = a
                        phasex()
                elif upto == 2:
                    with conv_pools() as a, conv_pools2() as b:
                        pools["pxps"], pools["pcps"] = a, b
                        merged_x_conv0()
                        for q in range(1, NQ):
                            conv_chunk(q)
                    # dummy tiny collective so interleaved benching with
                    # collective-bearing programs can't desync the mesh
                    dag_i = dram.tile([P, 2], fp16, name=f"dagi_{rep}")
                    dag_o = dram.tile([NCORES * P, 2], fp16,
                                      addr_space="Shared", name=f"dago_{rep}")
                    nc.scalar.dma_start(dag_i[:, :], spack[:])
                    nc.gpsimd.collective_compute(
                        "AllGather", ALU.bypass, replica_groups=rg,
                        ins=[dag_i.opt()], outs=[dag_o.opt()])
                    nc.sync.dma_start(spack[:], dag_o[0:P, :])
                elif upto == 3:
                    with conv_pools() as a, conv_pools2() as b:
                        pools["pxps"], pools["pcps"] = a, b
                        merged_x_conv0()
                        for q in range(1, NQ):
                            conv_chunk(q)
                        ag_bn()
                else:
                    whole()
    nc.compile()
    return nc


# --------------------------------------------------------------------------
# Host-side unsharding: unpack class-layout, mirror the missing triangle.
# --------------------------------------------------------------------------

def assemble_output(results):
    full = np.zeros((NPAD, NPAD), dtype=np.float32)
    filled = np.zeros((NBLK, NBLK), dtype=bool)
    for c in range(NCORES):
        o = results[c]["out"].astype(np.float32)  # [P, OUT_W] fp16 -> f32
        for k in range(NW):
            a = c * NW + k  # global row block
            slot = o[:, int(SLOT_OFF[k]):int(SLOT_OFF[k]) + SLOT_W[k]]
            w2 = len(CLS[k]) * 512
            for ci, m in enumerate(CLS[k]):
                for t in range(TPC):
                    b = t * NW + m  # global col block
                    half, tt = divmod(t, 4)
                    c0 = half * w2 + ci * 512 + tt * P
                    full[a * P:(a + 1) * P, b * P:(b + 1) * P] = \
                        slot[:, c0:c0 + P]
                    filled[a, b] = True
    for a in range(NBLK):
        for b in range(NBLK):
            if not filled[a, b]:
                full[a * P:(a + 1) * P, b * P:(b + 1) * P] = \
                    full[b * P:(b + 1) * P, a * P:(a + 1) * P].T
    return np.ascontiguousarray(full[:N, :N])


from concourse import bass_utils

_CACHE = {}


def kernel(x, edge_index, W, b, gamma, beta):
    in_maps, meta = preprocess(x, edge_index, W, gamma, beta)
    if "nc" not in _CACHE:
        _CACHE["nc"] = build(meta)
    nc = _CACHE["nc"]
    res = bass_utils.run_bass_kernel_spmd(
        nc, in_maps, core_ids=list(range(NCORES)))
    return assemble_output(res.results)
